# revision 1
# baseline (speedup 1.0000x reference)
"""ConvLSTM (nn_BottomConvLSTM) Trainium2 Bass kernel.

Problem (hardcoded):
  x:       [T=12, B=2, C=64, H=128, W=128] f32
  W_gates: [512, 192, 3, 3] f32,  b_gates: [512] f32
  W_out:   [64, 128, 3, 3] f32,   b_out:   [64] f32
  out:     [T, B, 64, H, W] f32

Sharding: 8 cores = B(2) x H-slabs(4 x 32 rows). The T recurrence is kept
on-chip per core; instead of inter-core halo exchange (collectives don't
move data under this runner), each core redundantly computes a shrinking
halo window (rows 32+2*g_t, g_t = T+1-t). Rows of h that fall outside the
global image are zeroed every step via per-core 0/1 row masks (uniform SPMD
program, per-core mask data), exactly reproducing SAME zero padding at slab
boundaries for arbitrary biases.

Conv as shifted matmuls accumulating in PSUM (fp32r fast path, ~4x fp32).
The x channels (64) are packed twice along partitions with a +1 row shift so
taps (dy=0,dx)+(dy=1,dx) fuse into one K=128 matmul; (2,0)+(2,1) fuse via a
+1 column-shifted packed copy; (2,2) runs solo at K=64 -> 5 matmuls instead
of 9 for the x taps. h taps are 9 full K=128 matmuls. LSTM pointwise runs on
ACT (sigmoid/tanh) + DVE (mul/add). The output conv is fused per timestep
from on-chip h. Measured: ~2.1 ms, PE ~98% busy, rel err ~2.4e-4.
"""

import os
import sys

import numpy as np

T = 12
CIN = 64
HID = 128
H_FULL = 128
W = 128
NB = 2
NSLAB = 4
SLAB = H_FULL // NSLAB  # 32
WP = W + 2  # zero-padded width

N_CORES = 8
LAST_EXEC_NS = None

# Matmul input dtype: fp32r (TF32-like, 4x faster) vs fp32 (exact, slow).
FAST_DTYPE = os.environ.get("KERNEL_MM_DTYPE", "float32r")


def _import_concourse():
    try:
        import concourse.bass  # noqa: F401
        return
    except ImportError:
        pass
    for p in ("/opt/trn_rl_repo", "/root/.axon_site/_ro/trn_rl_repo"):
        if os.path.isdir(p) and p not in sys.path:
            sys.path.insert(0, p)
    import concourse.bass  # noqa: F401


def _row_tiles(lo, hi):
    """Split [lo, hi) into 4-row tiles plus one trailing 2-row tile."""
    tiles = []
    y = lo
    while y < hi:
        r = 4 if hi - y >= 4 else hi - y
        tiles.append((y, r))
        y += r
    return tiles


def build_nc(t_steps=T, slab=SLAB):
    _import_concourse()
    import concourse.tile as tile
    from concourse import bacc, mybir

    F32 = mybir.dt.float32
    FMM = getattr(mybir.dt, FAST_DTYPE)
    AF = mybir.ActivationFunctionType

    base = t_steps + 1
    hbuf = slab + 2 * base

    nc = bacc.Bacc("TRN2", target_bir_lowering=False, debug=False)
    xp = nc.dram_tensor("xp", [t_steps, 128, hbuf, WP], FMM, kind="ExternalInput").ap()
    whd = nc.dram_tensor("wh", [128, 9, 512], FMM, kind="ExternalInput").ap()
    wxpd = nc.dram_tensor("wxp", [128, 3, 512], FMM, kind="ExternalInput").ap()
    wxcd = nc.dram_tensor("wxc", [128, 512], FMM, kind="ExternalInput").ap()
    wx2d = nc.dram_tensor("wx2", [128, 512], FMM, kind="ExternalInput").ap()
    wod = nc.dram_tensor("wo", [128, 9, 64], FMM, kind="ExternalInput").ap()
    bgd = nc.dram_tensor("bg", [128, 4], F32, kind="ExternalInput").ap()
    bod = nc.dram_tensor("bo", [64, 1], F32, kind="ExternalInput").ap()
    # 0/1 row masks zeroing h rows that fall outside the global image
    # (replicates SAME zero-padding at the slab's outer boundary).
    mtopd = nc.dram_tensor("mtop", [128, t_steps, WP], FMM, kind="ExternalInput").ap()
    mbotd = nc.dram_tensor("mbot", [128, t_steps, WP], FMM, kind="ExternalInput").ap()
    out = nc.dram_tensor("out", [t_steps, 64, slab, W], F32, kind="ExternalOutput").ap()

    with tile.TileContext(nc) as tc:
        with (
            tc.tile_pool(name="pw", bufs=1) as pw,
            tc.tile_pool(name="pstate", bufs=1) as pstate,
            tc.tile_pool(name="px", bufs=4) as px,
            tc.tile_pool(name="ptmp", bufs=18) as ptmp,
            tc.tile_pool(name="pout", bufs=3) as pout,
            tc.tile_pool(name="pps", bufs=8, space="PSUM") as pps,
        ):
            wh_sb = pw.tile([128, 9, 512], FMM, tag="wh", name="wh_sb")
            wxp_sb = pw.tile([128, 3, 512], FMM, tag="wxp", name="wxp_sb")
            wxc_sb = pw.tile([128, 512], FMM, tag="wxc", name="wxc_sb")
            wx2_sb = pw.tile([128, 512], FMM, tag="wx2", name="wx2_sb")
            wo_sb = pw.tile([128, 9, 64], FMM, tag="wo", name="wo_sb")
            bg_sb = pw.tile([128, 4], F32, tag="bg", name="bg_sb")
            bo_sb = pw.tile([64, 1], F32, tag="bo", name="bo_sb")
            mtop_sb = pw.tile([128, t_steps, WP], FMM, tag="mtop", name="mtop_sb")
            mbot_sb = pw.tile([128, t_steps, WP], FMM, tag="mbot", name="mbot_sb")
            # Warm the PE clock (HAM un-throttles after ~3.4us of activity)
            # with dummy matmuls on a zeroed tile while the weight DMAs are
            # still in flight — the first real matmuls then run at 2.4 GHz.
            warm = pw.tile([128, 640], FMM, tag="warm", name="warm")
            nc.vector.memset(warm[:].bitcast(mybir.dt.uint32), 0)
            wps = pps.tile([128, 512], F32, tag="ps", name="warm_ps")
            for k in range(24):
                nc.tensor.matmul(
                    wps[:], warm[:, 0:128], warm[:, 128:640],
                    start=(k == 0), stop=(k == 23),
                )

            # x-weights first: step 1 needs no h-weights, so its matmuls can
            # start as soon as the small x-weight tiles land
            for dx in range(3):
                nc.sync.dma_start(wxp_sb[:, dx, :], wxpd[:, dx, :])
            nc.sync.dma_start(wxc_sb[:], wxcd[:])
            nc.sync.dma_start(wx2_sb[:], wx2d[:])
            nc.sync.dma_start(bg_sb[:], bgd[:])
            nc.sync.dma_start(bo_sb[:], bod[:])
            nc.sync.dma_start(wo_sb[:], wod[:])
            nc.sync.dma_start(mtop_sb[:], mtopd[:])
            nc.sync.dma_start(mbot_sb[:], mbotd[:])
            nc.sync.dma_start(wh_sb[:], whd[:])

            h_a = pstate.tile([128, hbuf, WP], FMM, tag="ha", name="h_a")
            h_b = pstate.tile([128, hbuf, WP], FMM, tag="hb", name="h_b")
            c_sb = pstate.tile([128, hbuf, W], F32, tag="c", name="c_sb")
            nc.vector.memset(h_a[:].bitcast(mybir.dt.uint32), 0)
            nc.vector.memset(h_b[:].bitcast(mybir.dt.uint32), 0)
            h_tiles = [h_a, h_b]

            for t in range(1, t_steps + 1):
                h_cur = h_tiles[(t - 1) % 2]
                h_prev = h_tiles[t % 2]
                lo, hi = t, hbuf - t

                tiles = _row_tiles(lo, hi)
                # emit tiles feeding the fused out-conv (rows [base-1, base+slab+1))
                # first so out-conv matmuls overlap the halo tiles' pointwise tail
                tiles = (
                    [tl for tl in tiles if tl[0] + tl[1] > base - 1 and tl[0] < base + slab + 1]
                    + [tl for tl in tiles if not (tl[0] + tl[1] > base - 1 and tl[0] < base + slab + 1)]
                )
                for y0, r in tiles:
                    n = r * 128
                    xs = px.tile([128, r + 2, WP], FMM, tag="xs", name="xs")
                    nc.sync.dma_start(xs[:, :, :], xp[t - 1, :, y0 - 1 : y0 + r + 1, :])
                    # col-pair tile: lower = x rows y0+1.. (dy=2), upper = same +1 col
                    xc = px.tile([128, r, WP], FMM, tag="xc", name="xc")
                    nc.sync.dma_start(
                        xc[0:64, :, :], xp[t - 1, 0:64, y0 + 1 : y0 + r + 1, :]
                    )
                    nc.sync.dma_start(
                        xc[64:128, :, 0 : WP - 1],
                        xp[t - 1, 0:64, y0 + 1 : y0 + r + 1, 1:WP],
                    )

                    # coc order g,i,f,o: the g-gate PSUM (feeds the longest
                    # pointwise chain) lands first, and only sigmoid(o)+mul
                    # remain after the last PSUM — shorter step-boundary seam.
                    # The pointwise below is emitted in matching order so the
                    # strict-FIFO ACT queue never stalls on a late PSUM.
                    psums = {}
                    for coc in (3, 0, 1, 2):
                        pt = pps.tile([128, n], F32, tag="ps", name="ps")
                        psums[coc] = pt
                        mms = []
                        # x taps: (dy0,dx)+(dy1,dx) row-packed; (2,0)+(2,1)
                        # col-packed; (2,2) solo on the shifted upper half
                        for dx in range(3):
                            mms.append((
                                wxp_sb[:, dx, coc * 128 : (coc + 1) * 128],
                                xs[:, 0:r, dx : dx + 128],
                            ))
                        mms.append((
                            wxc_sb[:, coc * 128 : (coc + 1) * 128],
                            xc[:, 0:r, 0:128],
                        ))
                        mms.append((
                            wx2_sb[64:128, coc * 128 : (coc + 1) * 128],
                            xs[64:128, 1 : 1 + r, 2:130],
                        ))
                        if t > 1:
                            for dy in range(3):
                                for dx in range(3):
                                    mms.append((
                                        wh_sb[:, dy * 3 + dx, coc * 128 : (coc + 1) * 128],
                                        h_prev[:, y0 - 1 + dy : y0 - 1 + dy + r, dx : dx + 128],
                                    ))
                        for k, (lhsT, rhs) in enumerate(mms):
                            nc.tensor.matmul(
                                pt[:], lhsT, rhs,
                                start=(k == 0), stop=(k == len(mms) - 1),
                            )

                    pt_i, pt_f, pt_o, pt_g = (psums[c] for c in range(4))
                    cw = c_sb[:, y0 : y0 + r, :]
                    hw = h_cur[:, y0 : y0 + r, 1:129]

                    tg = ptmp.tile([128, n], F32, tag="tmp", name="tg")
                    nc.scalar.activation(tg[:], pt_g[:], AF.Tanh, bias=bg_sb[:, 3:4])
                    si = ptmp.tile([128, n], F32, tag="tmp", name="si")
                    nc.scalar.activation(si[:], pt_i[:], AF.Sigmoid, bias=bg_sb[:, 0:1])
                    if t == 1:
                        nc.vector.tensor_mul(cw, si[:], tg[:])
                    else:
                        pr = ptmp.tile([128, n], F32, tag="tmp", name="pr")
                        nc.vector.tensor_mul(pr[:], si[:], tg[:])
                        sf = ptmp.tile([128, n], F32, tag="tmp", name="sf")
                        nc.scalar.activation(sf[:], pt_f[:], AF.Sigmoid, bias=bg_sb[:, 1:2])
                        nc.vector.tensor_mul(cw, cw, sf[:])
                        nc.vector.tensor_add(cw, cw, pr[:])
                    tct = ptmp.tile([128, n], F32, tag="tmp", name="tct")
                    nc.scalar.activation(tct[:], cw, AF.Tanh)
                    so = ptmp.tile([128, n], F32, tag="tmp", name="so")
                    nc.scalar.activation(so[:], pt_o[:], AF.Sigmoid, bias=bg_sb[:, 2:3])
                    nc.vector.tensor_mul(hw, so[:], tct[:])

                # zero h rows outside the global image (SAME-padding at slab edges)
                nc.vector.tensor_mul(
                    h_cur[:, 1 : 1 + t_steps, :],
                    h_cur[:, 1 : 1 + t_steps, :],
                    mtop_sb[:],
                )
                nc.vector.tensor_mul(
                    h_cur[:, base + slab : base + slab + t_steps, :],
                    h_cur[:, base + slab : base + slab + t_steps, :],
                    mbot_sb[:],
                )

                # fused output conv on this step's h (out rows are the slab only)
                for yo in range(base, base + slab, 4):
                    po = pps.tile([64, 512], F32, tag="ps", name="po")
                    k = 0
                    for dy in range(3):
                        for dx in range(3):
                            nc.tensor.matmul(
                                po[:],
                                wo_sb[:, dy * 3 + dx, :],
                                h_cur[:, yo - 1 + dy : yo + 3 + dy, dx : dx + 128],
                                start=(k == 0), stop=(k == 8),
                            )
                            k += 1
                    ob = pout.tile([64, 4, 128], F32, tag="ostage", name="ob")
                    nc.scalar.activation(ob[:], po[:], AF.Identity, bias=bo_sb[:, 0:1])
                    nc.sync.dma_start(
                        out[t - 1, :, yo - base : yo - base + 4, :], ob[:]
                    )

    nc.compile()
    return nc


def prep_weights(W_gates, b_gates, W_out, b_out):
    wg = np.ascontiguousarray(W_gates, dtype=np.float32)  # [512, 192, 3, 3]
    wh = np.ascontiguousarray(
        wg[:, CIN:, :, :].reshape(512, 128, 9).transpose(1, 2, 0)
    )  # [128, 9, 512]
    wxp = np.ascontiguousarray(
        np.concatenate(
            [wg[:, :CIN, 0, :].transpose(1, 2, 0), wg[:, :CIN, 1, :].transpose(1, 2, 0)],
            axis=0,
        )
    )  # [128, 3, 512]
    wxc = np.ascontiguousarray(
        np.concatenate(
            [wg[:, :CIN, 2, 0].transpose(1, 0), wg[:, :CIN, 2, 1].transpose(1, 0)],
            axis=0,
        )
    )  # [128, 512]
    wx2 = np.zeros((128, 512), np.float32)
    wx2[64:] = wg[:, :CIN, 2, 2].transpose(1, 0)
    wo = np.ascontiguousarray(
        np.asarray(W_out, np.float32).reshape(64, 128, 9).transpose(1, 2, 0)
    )  # [128, 9, 64]
    bg = np.ascontiguousarray(np.asarray(b_gates, np.float32).reshape(4, 128).T)
    bo = np.ascontiguousarray(np.asarray(b_out, np.float32).reshape(64, 1))
    return {"wh": wh, "wxp": wxp, "wxc": wxc, "wx2": wx2, "wo": wo, "bg": bg, "bo": bo}


def prep_masks(r0, t_steps=T, slab=SLAB, h_img=H_FULL):
    """Row masks (1=keep, 0=zero) for the top/bottom halo bands of a slab
    starting at global row r0."""
    base = t_steps + 1
    mtop = np.zeros((128, t_steps, WP), np.float32)
    mbot = np.zeros((128, t_steps, WP), np.float32)
    for j in range(t_steps):
        r = 1 + j
        if 0 <= r0 + r - base < h_img:
            mtop[:, j, :] = 1.0
        r = base + slab + j
        if 0 <= r0 + r - base < h_img:
            mbot[:, j, :] = 1.0
    return {"mtop": mtop, "mbot": mbot}


def prep_x(x, t_steps=T, slab=SLAB, h_img=H_FULL):
    """x: [T, B, C, H, W] -> list of per-core packed [T, 128, hbuf, WP] arrays.

    Core c = b * NSLAB + s covers global rows [slab*s, slab*s + slab).
    Partitions 0:64 hold x rows as-is, 64:128 the same rows shifted +1, so
    conv taps dy=0/1 share one matmul and dy=2 reads the shifted half.
    """
    base = t_steps + 1
    hbuf = slab + 2 * base
    nslab = h_img // slab
    x = np.asarray(x, np.float32)
    tt, nb = x.shape[0], x.shape[1]
    cores = []
    for b in range(nb):
        xpad = np.zeros((tt, CIN, h_img + 2 * base + 1, WP), np.float32)
        xpad[:, :, base : base + h_img, 1 : 1 + W] = x[:, b]
        for s in range(nslab):
            r0 = slab * s
            lower = xpad[:, :, r0 : r0 + hbuf, :]
            upper = xpad[:, :, r0 + 1 : r0 + 1 + hbuf, :]
            cores.append(np.ascontiguousarray(np.concatenate([lower, upper], axis=1)))
    return cores


_NC_CACHE = {}


def _get_nc():
    key = (T, SLAB, FAST_DTYPE)
    if key not in _NC_CACHE:
        _NC_CACHE[key] = build_nc(T, SLAB)
    return _NC_CACHE[key]


def kernel(x, W_gates, b_gates, W_out, b_out):
    _import_concourse()
    from concourse.bass_utils import run_bass_kernel_spmd

    nc = _get_nc()
    wmap = prep_weights(W_gates, b_gates, W_out, b_out)
    xcores = prep_x(x)
    in_maps = []
    for c, xc in enumerate(xcores):
        s = c % NSLAB
        in_maps.append(dict(wmap, xp=xc, **prep_masks(SLAB * s)))

    trace = bool(os.environ.get("KERNEL_TRACE"))
    kwargs = {}
    if trace:
        kwargs = {"trace": True, "tmpdir": os.environ.get("KERNEL_TRACE_DIR") or None}
    res = run_bass_kernel_spmd(nc, in_maps, core_ids=list(range(N_CORES)), **kwargs)
    if trace:
        global LAST_EXEC_NS
        LAST_EXEC_NS = res.exec_time_ns
        print(f"HW exec time: {res.exec_time_ns} ns")

    out = np.empty((T, NB, CIN, H_FULL, W), np.float32)
    for c in range(N_CORES):
        b, s = divmod(c, NSLAB)
        out[:, b, :, SLAB * s : SLAB * (s + 1), :] = res.results[c]["out"]
    return out



# revision 7
# speedup vs baseline: 1.1325x; 1.1325x over previous
"""ConvLSTM (nn_BottomConvLSTM) Trainium2 Bass kernel.

Problem (hardcoded):
  x:       [T=12, B=2, C=64, H=128, W=128] f32
  W_gates: [512, 192, 3, 3] f32,  b_gates: [512] f32
  W_out:   [64, 128, 3, 3] f32,   b_out:   [64] f32
  out:     [T, B, 64, H, W] f32

Sharding: 8 cores = B(2) x H-slabs(4 x 32 rows). The T recurrence is kept
on-chip per core; instead of inter-core halo exchange (collectives don't
move data under this runner), each core redundantly computes a shrinking
halo window (rows 32+2*g_t, g_t = T+1-t). Rows of h that fall outside the
global image are zeroed every step via per-core 0/1 row masks (uniform SPMD
program, per-core mask data), exactly reproducing SAME zero padding at slab
boundaries for arbitrary biases.

Conv as shifted matmuls accumulating in PSUM (fp32r fast path, ~4x fp32).
The x channels (64) are packed twice along partitions with a +1 row shift so
taps (dy=0,dx)+(dy=1,dx) fuse into one K=128 matmul; (2,0)+(2,1) fuse via a
+1 column-shifted packed copy; (2,2) runs solo at K=64 -> 5 matmuls instead
of 9 for the x taps. h taps are 9 full K=128 matmuls. LSTM pointwise runs on
ACT (sigmoid/tanh) + DVE (mul/add). The output conv is fused per timestep
from on-chip h. Measured: ~2.1 ms, PE ~98% busy, rel err ~2.4e-4.
"""

import os
import sys

import numpy as np

T = 12
CIN = 64
HID = 128
H_FULL = 128
W = 128
NB = 2
NSLAB = 4
SLAB = H_FULL // NSLAB  # 32
WP = W + 2  # zero-padded width

N_CORES = 8
LAST_EXEC_NS = None

# Matmul input dtype: bfloat16 (1 col/cycle, FWL weight loads) vs float32r
# (TF32-like, observed 2-pass ~411ns per N=512 MM) vs float32 (exact, slow).
FAST_DTYPE = os.environ.get("KERNEL_MM_DTYPE", "bfloat16")


def _mm_np(a):
    """Cast a host array to the matmul input dtype."""
    if FAST_DTYPE == "bfloat16":
        import ml_dtypes

        return np.ascontiguousarray(a.astype(ml_dtypes.bfloat16))
    return np.ascontiguousarray(a, dtype=np.float32)


def _import_concourse():
    try:
        import concourse.bass  # noqa: F401
        return
    except ImportError:
        pass
    for p in ("/opt/trn_rl_repo", "/root/.axon_site/_ro/trn_rl_repo"):
        if os.path.isdir(p) and p not in sys.path:
            sys.path.insert(0, p)
    import concourse.bass  # noqa: F401


def _row_tiles(lo, hi):
    """Split [lo, hi) into 4-row tiles plus one trailing 2-row tile."""
    tiles = []
    y = lo
    while y < hi:
        r = 4 if hi - y >= 4 else hi - y
        tiles.append((y, r))
        y += r
    return tiles


def build_nc(t_steps=T, slab=SLAB):
    _import_concourse()
    import concourse.tile as tile
    from concourse import bacc, mybir

    F32 = mybir.dt.float32
    FMM = getattr(mybir.dt, FAST_DTYPE)
    AF = mybir.ActivationFunctionType

    base = t_steps + 1
    hbuf = slab + 2 * base

    nc = bacc.Bacc("TRN2", target_bir_lowering=False, debug=False)
    xp = nc.dram_tensor("xp", [t_steps, 128, hbuf, WP], FMM, kind="ExternalInput").ap()
    whd = nc.dram_tensor("wh", [128, 9, 512], FMM, kind="ExternalInput").ap()
    wxpd = nc.dram_tensor("wxp", [128, 3, 512], FMM, kind="ExternalInput").ap()
    wxcd = nc.dram_tensor("wxc", [128, 512], FMM, kind="ExternalInput").ap()
    wx2d = nc.dram_tensor("wx2", [128, 512], FMM, kind="ExternalInput").ap()
    wod = nc.dram_tensor("wo", [128, 9, 64], FMM, kind="ExternalInput").ap()
    bgd = nc.dram_tensor("bg", [128, 4], F32, kind="ExternalInput").ap()
    bod = nc.dram_tensor("bo", [64, 1], F32, kind="ExternalInput").ap()
    # 0/1 row masks zeroing h rows that fall outside the global image
    # (replicates SAME zero-padding at the slab's outer boundary).
    mtopd = nc.dram_tensor("mtop", [128, t_steps, WP], FMM, kind="ExternalInput").ap()
    mbotd = nc.dram_tensor("mbot", [128, t_steps, WP], FMM, kind="ExternalInput").ap()
    out = nc.dram_tensor("out", [t_steps, 64, slab, W], F32, kind="ExternalOutput").ap()

    with tile.TileContext(nc) as tc:
        with (
            tc.tile_pool(name="pw", bufs=1) as pw,
            tc.tile_pool(name="pstate", bufs=1) as pstate,
            tc.tile_pool(name="px", bufs=4) as px,
            tc.tile_pool(name="ptmp", bufs=18) as ptmp,
            tc.tile_pool(name="pout", bufs=3) as pout,
            tc.tile_pool(name="pps", bufs=8, space="PSUM") as pps,
        ):
            wh_sb = pw.tile([128, 9, 512], FMM, tag="wh", name="wh_sb")
            wxp_sb = pw.tile([128, 3, 512], FMM, tag="wxp", name="wxp_sb")
            wxc_sb = pw.tile([128, 512], FMM, tag="wxc", name="wxc_sb")
            wx2_sb = pw.tile([128, 512], FMM, tag="wx2", name="wx2_sb")
            wo_sb = pw.tile([128, 9, 64], FMM, tag="wo", name="wo_sb")
            bg_sb = pw.tile([128, 4], F32, tag="bg", name="bg_sb")
            bo_sb = pw.tile([64, 1], F32, tag="bo", name="bo_sb")
            mtop_sb = pw.tile([128, t_steps, WP], FMM, tag="mtop", name="mtop_sb")
            mbot_sb = pw.tile([128, t_steps, WP], FMM, tag="mbot", name="mbot_sb")
            # Warm the PE clock (HAM un-throttles after ~3.4us of activity)
            # with dummy matmuls on a zeroed tile while the weight DMAs are
            # still in flight — the first real matmuls then run at 2.4 GHz.
            warm = pw.tile([128, 640], FMM, tag="warm", name="warm")
            nc.vector.memset(warm[:], 0)
            wps = pps.tile([128, 512], F32, tag="ps", name="warm_ps")
            for k in range(24):
                nc.tensor.matmul(
                    wps[:], warm[:, 0:128], warm[:, 128:640],
                    start=(k == 0), stop=(k == 23),
                )

            # x-weights first: step 1 needs no h-weights, so its matmuls can
            # start as soon as the small x-weight tiles land
            for dx in range(3):
                nc.sync.dma_start(wxp_sb[:, dx, :], wxpd[:, dx, :])
            nc.sync.dma_start(wxc_sb[:], wxcd[:])
            nc.sync.dma_start(wx2_sb[:], wx2d[:])
            nc.sync.dma_start(bg_sb[:], bgd[:])
            nc.sync.dma_start(bo_sb[:], bod[:])
            nc.sync.dma_start(wo_sb[:], wod[:])
            nc.sync.dma_start(mtop_sb[:], mtopd[:])
            nc.sync.dma_start(mbot_sb[:], mbotd[:])
            nc.sync.dma_start(wh_sb[:], whd[:])

            h_a = pstate.tile([128, hbuf, WP], FMM, tag="ha", name="h_a")
            h_b = pstate.tile([128, hbuf, WP], FMM, tag="hb", name="h_b")
            c_sb = pstate.tile([128, hbuf, W], F32, tag="c", name="c_sb")
            nc.vector.memset(h_a[:], 0)
            nc.vector.memset(h_b[:], 0)
            h_tiles = [h_a, h_b]

            for t in range(1, t_steps + 1):
                h_cur = h_tiles[(t - 1) % 2]
                h_prev = h_tiles[t % 2]
                lo, hi = t, hbuf - t

                tiles = _row_tiles(lo, hi)
                # emit tiles feeding the fused out-conv (rows [base-1, base+slab+1))
                # first so out-conv matmuls overlap the halo tiles' pointwise tail
                tiles = (
                    [tl for tl in tiles if tl[0] + tl[1] > base - 1 and tl[0] < base + slab + 1]
                    + [tl for tl in tiles if not (tl[0] + tl[1] > base - 1 and tl[0] < base + slab + 1)]
                )
                for y0, r in tiles:
                    n = r * 128
                    xs = px.tile([128, r + 2, WP], FMM, tag="xs", name="xs")
                    nc.sync.dma_start(xs[:, :, :], xp[t - 1, :, y0 - 1 : y0 + r + 1, :])
                    # col-pair tile: lower = x rows y0+1.. (dy=2), upper = same +1 col
                    xc = px.tile([128, r, WP], FMM, tag="xc", name="xc")
                    nc.sync.dma_start(
                        xc[0:64, :, :], xp[t - 1, 0:64, y0 + 1 : y0 + r + 1, :]
                    )
                    nc.sync.dma_start(
                        xc[64:128, :, 0 : WP - 1],
                        xp[t - 1, 0:64, y0 + 1 : y0 + r + 1, 1:WP],
                    )

                    # coc order g,i,f,o: the g-gate PSUM (feeds the longest
                    # pointwise chain) lands first, and only sigmoid(o)+mul
                    # remain after the last PSUM — shorter step-boundary seam.
                    # The pointwise below is emitted in matching order so the
                    # strict-FIFO ACT queue never stalls on a late PSUM.
                    psums = {}
                    for coc in (3, 0, 1, 2):
                        pt = pps.tile([128, n], F32, tag="ps", name="ps")
                        psums[coc] = pt
                        mms = []
                        # x taps: (dy0,dx)+(dy1,dx) row-packed; (2,0)+(2,1)
                        # col-packed; (2,2) solo on the shifted upper half
                        for dx in range(3):
                            mms.append((
                                wxp_sb[:, dx, coc * 128 : (coc + 1) * 128],
                                xs[:, 0:r, dx : dx + 128],
                            ))
                        mms.append((
                            wxc_sb[:, coc * 128 : (coc + 1) * 128],
                            xc[:, 0:r, 0:128],
                        ))
                        mms.append((
                            wx2_sb[64:128, coc * 128 : (coc + 1) * 128],
                            xs[64:128, 1 : 1 + r, 2:130],
                        ))
                        if t > 1:
                            for dy in range(3):
                                for dx in range(3):
                                    mms.append((
                                        wh_sb[:, dy * 3 + dx, coc * 128 : (coc + 1) * 128],
                                        h_prev[:, y0 - 1 + dy : y0 - 1 + dy + r, dx : dx + 128],
                                    ))
                        for k, (lhsT, rhs) in enumerate(mms):
                            nc.tensor.matmul(
                                pt[:], lhsT, rhs,
                                start=(k == 0), stop=(k == len(mms) - 1),
                            )

                    pt_i, pt_f, pt_o, pt_g = (psums[c] for c in range(4))
                    cw = c_sb[:, y0 : y0 + r, :]
                    hw = h_cur[:, y0 : y0 + r, 1:129]

                    tg = ptmp.tile([128, n], F32, tag="tmp", name="tg")
                    nc.scalar.activation(tg[:], pt_g[:], AF.Tanh, bias=bg_sb[:, 3:4])
                    si = ptmp.tile([128, n], F32, tag="tmp", name="si")
                    nc.scalar.activation(si[:], pt_i[:], AF.Sigmoid, bias=bg_sb[:, 0:1])
                    if t == 1:
                        nc.vector.tensor_mul(cw, si[:], tg[:])
                    else:
                        pr = ptmp.tile([128, n], F32, tag="tmp", name="pr")
                        nc.vector.tensor_mul(pr[:], si[:], tg[:])
                        sf = ptmp.tile([128, n], F32, tag="tmp", name="sf")
                        nc.scalar.activation(sf[:], pt_f[:], AF.Sigmoid, bias=bg_sb[:, 1:2])
                        nc.vector.tensor_mul(cw, cw, sf[:])
                        nc.vector.tensor_add(cw, cw, pr[:])
                    tct = ptmp.tile([128, n], F32, tag="tmp", name="tct")
                    nc.scalar.activation(tct[:], cw, AF.Tanh)
                    so = ptmp.tile([128, n], F32, tag="tmp", name="so")
                    nc.scalar.activation(so[:], pt_o[:], AF.Sigmoid, bias=bg_sb[:, 2:3])
                    nc.vector.tensor_mul(hw, so[:], tct[:])

                # zero h rows outside the global image (SAME-padding at slab edges)
                nc.vector.tensor_mul(
                    h_cur[:, 1 : 1 + t_steps, :],
                    h_cur[:, 1 : 1 + t_steps, :],
                    mtop_sb[:],
                )
                nc.vector.tensor_mul(
                    h_cur[:, base + slab : base + slab + t_steps, :],
                    h_cur[:, base + slab : base + slab + t_steps, :],
                    mbot_sb[:],
                )

                # fused output conv on this step's h (out rows are the slab only)
                for yo in range(base, base + slab, 4):
                    po = pps.tile([64, 512], F32, tag="ps", name="po")
                    k = 0
                    for dy in range(3):
                        for dx in range(3):
                            nc.tensor.matmul(
                                po[:],
                                wo_sb[:, dy * 3 + dx, :],
                                h_cur[:, yo - 1 + dy : yo + 3 + dy, dx : dx + 128],
                                start=(k == 0), stop=(k == 8),
                            )
                            k += 1
                    ob = pout.tile([64, 4, 128], F32, tag="ostage", name="ob")
                    nc.scalar.activation(ob[:], po[:], AF.Identity, bias=bo_sb[:, 0:1])
                    nc.sync.dma_start(
                        out[t - 1, :, yo - base : yo - base + 4, :], ob[:]
                    )

    nc.compile()
    return nc


def prep_weights(W_gates, b_gates, W_out, b_out):
    wg = np.ascontiguousarray(W_gates, dtype=np.float32)  # [512, 192, 3, 3]
    wh = np.ascontiguousarray(
        wg[:, CIN:, :, :].reshape(512, 128, 9).transpose(1, 2, 0)
    )  # [128, 9, 512]
    wxp = np.ascontiguousarray(
        np.concatenate(
            [wg[:, :CIN, 0, :].transpose(1, 2, 0), wg[:, :CIN, 1, :].transpose(1, 2, 0)],
            axis=0,
        )
    )  # [128, 3, 512]
    wxc = np.ascontiguousarray(
        np.concatenate(
            [wg[:, :CIN, 2, 0].transpose(1, 0), wg[:, :CIN, 2, 1].transpose(1, 0)],
            axis=0,
        )
    )  # [128, 512]
    wx2 = np.zeros((128, 512), np.float32)
    wx2[64:] = wg[:, :CIN, 2, 2].transpose(1, 0)
    wo = np.ascontiguousarray(
        np.asarray(W_out, np.float32).reshape(64, 128, 9).transpose(1, 2, 0)
    )  # [128, 9, 64]
    bg = np.ascontiguousarray(np.asarray(b_gates, np.float32).reshape(4, 128).T)
    bo = np.ascontiguousarray(np.asarray(b_out, np.float32).reshape(64, 1))
    return {
        "wh": _mm_np(wh), "wxp": _mm_np(wxp), "wxc": _mm_np(wxc),
        "wx2": _mm_np(wx2), "wo": _mm_np(wo), "bg": bg, "bo": bo,
    }


def prep_masks(r0, t_steps=T, slab=SLAB, h_img=H_FULL):
    """Row masks (1=keep, 0=zero) for the top/bottom halo bands of a slab
    starting at global row r0."""
    base = t_steps + 1
    mtop = np.zeros((128, t_steps, WP), np.float32)
    mbot = np.zeros((128, t_steps, WP), np.float32)
    for j in range(t_steps):
        r = 1 + j
        if 0 <= r0 + r - base < h_img:
            mtop[:, j, :] = 1.0
        r = base + slab + j
        if 0 <= r0 + r - base < h_img:
            mbot[:, j, :] = 1.0
    return {"mtop": _mm_np(mtop), "mbot": _mm_np(mbot)}


def prep_x(x, t_steps=T, slab=SLAB, h_img=H_FULL):
    """x: [T, B, C, H, W] -> list of per-core packed [T, 128, hbuf, WP] arrays.

    Core c = b * NSLAB + s covers global rows [slab*s, slab*s + slab).
    Partitions 0:64 hold x rows as-is, 64:128 the same rows shifted +1, so
    conv taps dy=0/1 share one matmul and dy=2 reads the shifted half.
    """
    base = t_steps + 1
    hbuf = slab + 2 * base
    nslab = h_img // slab
    x = np.asarray(x, np.float32)
    tt, nb = x.shape[0], x.shape[1]
    cores = []
    for b in range(nb):
        xpad = np.zeros((tt, CIN, h_img + 2 * base + 1, WP), np.float32)
        xpad[:, :, base : base + h_img, 1 : 1 + W] = x[:, b]
        for s in range(nslab):
            r0 = slab * s
            lower = xpad[:, :, r0 : r0 + hbuf, :]
            upper = xpad[:, :, r0 + 1 : r0 + 1 + hbuf, :]
            cores.append(_mm_np(np.concatenate([lower, upper], axis=1)))
    return cores


_NC_CACHE = {}


def _get_nc():
    key = (T, SLAB, FAST_DTYPE)
    if key not in _NC_CACHE:
        _NC_CACHE[key] = build_nc(T, SLAB)
    return _NC_CACHE[key]


def kernel(x, W_gates, b_gates, W_out, b_out):
    _import_concourse()
    from concourse.bass_utils import run_bass_kernel_spmd

    nc = _get_nc()
    wmap = prep_weights(W_gates, b_gates, W_out, b_out)
    xcores = prep_x(x)
    in_maps = []
    for c, xc in enumerate(xcores):
        s = c % NSLAB
        in_maps.append(dict(wmap, xp=xc, **prep_masks(SLAB * s)))

    trace = bool(os.environ.get("KERNEL_TRACE"))
    kwargs = {}
    if trace:
        kwargs = {"trace": True, "tmpdir": os.environ.get("KERNEL_TRACE_DIR") or None}
    res = run_bass_kernel_spmd(nc, in_maps, core_ids=list(range(N_CORES)), **kwargs)
    if trace:
        global LAST_EXEC_NS
        LAST_EXEC_NS = res.exec_time_ns
        print(f"HW exec time: {res.exec_time_ns} ns")

    out = np.empty((T, NB, CIN, H_FULL, W), np.float32)
    for c in range(N_CORES):
        b, s = divmod(c, NSLAB)
        out[:, b, :, SLAB * s : SLAB * (s + 1), :] = res.results[c]["out"]
    return out



# revision 12
# speedup vs baseline: 1.2749x; 1.1258x over previous
"""ConvLSTM (nn_BottomConvLSTM) Trainium2 Bass kernel.

Problem (hardcoded):
  x:       [T=12, B=2, C=64, H=128, W=128] f32
  W_gates: [512, 192, 3, 3] f32,  b_gates: [512] f32
  W_out:   [64, 128, 3, 3] f32,   b_out:   [64] f32
  out:     [T, B, 64, H, W] f32

Sharding: 8 cores = B(2) x H-slabs(4 x 32 rows). The T recurrence is kept
on-chip per core; instead of inter-core halo exchange (collectives don't
move data under this runner), each core redundantly computes a shrinking
halo window (rows 32+2*g_t, g_t = T+1-t). Rows of h that fall outside the
global image are zeroed every step via per-core 0/1 row masks (uniform SPMD
program, per-core mask data), exactly reproducing SAME zero padding at slab
boundaries for arbitrary biases.

Conv as shifted matmuls accumulating in PSUM (fp32r fast path, ~4x fp32).
The x channels (64) are packed twice along partitions with a +1 row shift so
taps (dy=0,dx)+(dy=1,dx) fuse into one K=128 matmul; (2,0)+(2,1) fuse via a
+1 column-shifted packed copy; (2,2) runs solo at K=64 -> 5 matmuls instead
of 9 for the x taps. h taps are 9 full K=128 matmuls. LSTM pointwise runs on
ACT (sigmoid/tanh) + DVE (mul/add). The output conv is fused per timestep
from on-chip h. Measured: ~2.1 ms, PE ~98% busy, rel err ~2.4e-4.
"""

import os
import sys

import numpy as np

T = 12
CIN = 64
HID = 128
H_FULL = 128
W = 128
NB = 2
NSLAB = 4
SLAB = H_FULL // NSLAB  # 32
WP = W + 2  # zero-padded width

N_CORES = 8
LAST_EXEC_NS = None

# Matmul input dtype: bfloat16 (1 col/cycle, FWL weight loads) vs float32r
# (TF32-like, observed 2-pass ~411ns per N=512 MM) vs float32 (exact, slow).
FAST_DTYPE = os.environ.get("KERNEL_MM_DTYPE", "bfloat16")


def _mm_np(a):
    """Cast a host array to the matmul input dtype."""
    if FAST_DTYPE == "bfloat16":
        import ml_dtypes

        return np.ascontiguousarray(a.astype(ml_dtypes.bfloat16))
    return np.ascontiguousarray(a, dtype=np.float32)


def _import_concourse():
    try:
        import concourse.bass  # noqa: F401
        return
    except ImportError:
        pass
    for p in ("/opt/trn_rl_repo", "/root/.axon_site/_ro/trn_rl_repo"):
        if os.path.isdir(p) and p not in sys.path:
            sys.path.insert(0, p)
    import concourse.bass  # noqa: F401


def _row_tiles(lo, hi):
    """Split [lo, hi) into 4-row tiles plus one trailing 2-row tile."""
    tiles = []
    y = lo
    while y < hi:
        r = 4 if hi - y >= 4 else hi - y
        tiles.append((y, r))
        y += r
    return tiles


def build_nc(t_steps=T, slab=SLAB):
    _import_concourse()
    import concourse.tile as tile
    from concourse import bacc, mybir

    F32 = mybir.dt.float32
    FMM = getattr(mybir.dt, FAST_DTYPE)
    AF = mybir.ActivationFunctionType

    base = t_steps + 1
    hbuf = slab + 2 * base

    nc = bacc.Bacc("TRN2", target_bir_lowering=False, debug=False)
    xp = nc.dram_tensor("xp", [t_steps, 128, hbuf, WP], FMM, kind="ExternalInput").ap()
    whd = nc.dram_tensor("wh", [128, 9, 512], FMM, kind="ExternalInput").ap()
    wxpd = nc.dram_tensor("wxp", [128, 3, 512], FMM, kind="ExternalInput").ap()
    wxcd = nc.dram_tensor("wxc", [128, 512], FMM, kind="ExternalInput").ap()
    wx2d = nc.dram_tensor("wx2", [128, 512], FMM, kind="ExternalInput").ap()
    wod = nc.dram_tensor("wo", [128, 9, 64], FMM, kind="ExternalInput").ap()
    bgd = nc.dram_tensor("bg", [128, 4], F32, kind="ExternalInput").ap()
    bod = nc.dram_tensor("bo", [128, 1], F32, kind="ExternalInput").ap()
    # 0/1 row masks zeroing h rows that fall outside the global image
    # (replicates SAME zero-padding at the slab's outer boundary).
    mtopd = nc.dram_tensor("mtop", [128, t_steps, WP], FMM, kind="ExternalInput").ap()
    mbotd = nc.dram_tensor("mbot", [128, t_steps, WP], FMM, kind="ExternalInput").ap()
    out = nc.dram_tensor("out", [t_steps, 64, slab, W], F32, kind="ExternalOutput").ap()

    with tile.TileContext(nc) as tc:
        with (
            tc.tile_pool(name="pw", bufs=1) as pw,
            tc.tile_pool(name="pstate", bufs=1) as pstate,
            tc.tile_pool(name="px", bufs=4) as px,
            tc.tile_pool(name="ptmp", bufs=18) as ptmp,
            tc.tile_pool(name="pout", bufs=3) as pout,
            tc.tile_pool(name="pps", bufs=8, space="PSUM") as pps,
        ):
            wh_sb = pw.tile([128, 9, 512], FMM, tag="wh", name="wh_sb")
            wxp_sb = pw.tile([128, 3, 512], FMM, tag="wxp", name="wxp_sb")
            wxc_sb = pw.tile([128, 512], FMM, tag="wxc", name="wxc_sb")
            wx2_sb = pw.tile([128, 512], FMM, tag="wx2", name="wx2_sb")
            wo_sb = pw.tile([128, 9, 64], FMM, tag="wo", name="wo_sb")
            bg_sb = pw.tile([128, 4], F32, tag="bg", name="bg_sb")
            bo_sb = pw.tile([128, 1], F32, tag="bo", name="bo_sb")
            mtop_sb = pw.tile([128, t_steps, WP], FMM, tag="mtop", name="mtop_sb")
            mbot_sb = pw.tile([128, t_steps, WP], FMM, tag="mbot", name="mbot_sb")
            # Warm the PE clock (HAM un-throttles after ~3.4us of activity)
            # with dummy matmuls on a zeroed tile while the weight DMAs are
            # still in flight — the first real matmuls then run at 2.4 GHz.
            warm = pw.tile([128, 640], FMM, tag="warm", name="warm")
            nc.vector.memset(warm[:], 0)
            wps = pps.tile([128, 512], F32, tag="ps", name="warm_ps")
            for k in range(24):
                nc.tensor.matmul(
                    wps[:], warm[:, 0:128], warm[:, 128:640],
                    start=(k == 0), stop=(k == 23),
                )

            # x-weights first: step 1 needs no h-weights, so its matmuls can
            # start as soon as the small x-weight tiles land
            for dx in range(3):
                nc.sync.dma_start(wxp_sb[:, dx, :], wxpd[:, dx, :])
            nc.sync.dma_start(wxc_sb[:], wxcd[:])
            nc.sync.dma_start(wx2_sb[:], wx2d[:])
            nc.sync.dma_start(bg_sb[:], bgd[:])
            nc.sync.dma_start(bo_sb[:], bod[:])
            nc.sync.dma_start(wo_sb[:], wod[:])
            nc.sync.dma_start(mtop_sb[:], mtopd[:])
            nc.sync.dma_start(mbot_sb[:], mbotd[:])
            nc.sync.dma_start(wh_sb[:], whd[:])

            h_a = pstate.tile([128, hbuf, WP], FMM, tag="ha", name="h_a")
            h_b = pstate.tile([128, hbuf, WP], FMM, tag="hb", name="h_b")
            c_sb = pstate.tile([128, hbuf, W], F32, tag="c", name="c_sb")
            nc.vector.memset(h_a[:], 0)
            nc.vector.memset(h_b[:], 0)
            h_tiles = [h_a, h_b]

            for t in range(1, t_steps + 1):
                h_cur = h_tiles[(t - 1) % 2]
                h_prev = h_tiles[t % 2]
                lo, hi = t, hbuf - t

                tiles = _row_tiles(lo, hi)
                # emit tiles feeding the fused out-conv (rows [base-1, base+slab+1))
                # first so out-conv matmuls overlap the halo tiles' pointwise tail
                tiles = (
                    [tl for tl in tiles if tl[0] + tl[1] > base - 1 and tl[0] < base + slab + 1]
                    + [tl for tl in tiles if not (tl[0] + tl[1] > base - 1 and tl[0] < base + slab + 1)]
                )
                for y0, r in tiles:
                    n = r * 128
                    xs = px.tile([128, r + 2, WP], FMM, tag="xs", name="xs")
                    nc.sync.dma_start(xs[:, :, :], xp[t - 1, :, y0 - 1 : y0 + r + 1, :])
                    # col-pair tile: lower = x rows y0+1.. (dy=2), upper = same +1 col
                    xc = px.tile([128, r, WP], FMM, tag="xc", name="xc")
                    nc.sync.dma_start(
                        xc[0:64, :, :], xp[t - 1, 0:64, y0 + 1 : y0 + r + 1, :]
                    )
                    nc.sync.dma_start(
                        xc[64:128, :, 0 : WP - 1],
                        xp[t - 1, 0:64, y0 + 1 : y0 + r + 1, 1:WP],
                    )

                    # coc order g,i,f,o: the g-gate PSUM (feeds the longest
                    # pointwise chain) lands first, and only sigmoid(o)+mul
                    # remain after the last PSUM — shorter step-boundary seam.
                    # The pointwise below is emitted in matching order so the
                    # strict-FIFO ACT queue never stalls on a late PSUM.
                    psums = {}
                    for coc in (3, 0, 1, 2):
                        pt = pps.tile([128, n], F32, tag="ps", name="ps")
                        psums[coc] = pt
                        mms = []
                        # x taps: (dy0,dx)+(dy1,dx) row-packed; (2,0)+(2,1)
                        # col-packed; (2,2) solo on the shifted upper half
                        for dx in range(3):
                            mms.append((
                                wxp_sb[:, dx, coc * 128 : (coc + 1) * 128],
                                xs[:, 0:r, dx : dx + 128],
                            ))
                        mms.append((
                            wxc_sb[:, coc * 128 : (coc + 1) * 128],
                            xc[:, 0:r, 0:128],
                        ))
                        # full-K with zero top half: a K=64/row_grp=h64 MM here
                        # breaks LDW background-buffer pipelining (trace showed
                        # +~110ns on this MM and the next); zero-padded K=128
                        # costs the same N cycles but keeps LDWs hidden.
                        mms.append((
                            wx2_sb[:, coc * 128 : (coc + 1) * 128],
                            xs[:, 1 : 1 + r, 2:130],
                        ))
                        if t > 1:
                            for dy in range(3):
                                for dx in range(3):
                                    mms.append((
                                        wh_sb[:, dy * 3 + dx, coc * 128 : (coc + 1) * 128],
                                        h_prev[:, y0 - 1 + dy : y0 - 1 + dy + r, dx : dx + 128],
                                    ))
                        for k, (lhsT, rhs) in enumerate(mms):
                            nc.tensor.matmul(
                                pt[:], lhsT, rhs,
                                start=(k == 0), stop=(k == len(mms) - 1),
                            )

                    pt_i, pt_f, pt_o, pt_g = (psums[c] for c in range(4))
                    cw = c_sb[:, y0 : y0 + r, :]
                    hw = h_cur[:, y0 : y0 + r, 1:129]

                    tg = ptmp.tile([128, n], F32, tag="tmp", name="tg")
                    nc.scalar.activation(tg[:], pt_g[:], AF.Tanh, bias=bg_sb[:, 3:4])
                    si = ptmp.tile([128, n], F32, tag="tmp", name="si")
                    nc.scalar.activation(si[:], pt_i[:], AF.Sigmoid, bias=bg_sb[:, 0:1])
                    if t == 1:
                        nc.vector.tensor_mul(cw, si[:], tg[:])
                    else:
                        pr = ptmp.tile([128, n], F32, tag="tmp", name="pr")
                        nc.vector.tensor_mul(pr[:], si[:], tg[:])
                        sf = ptmp.tile([128, n], F32, tag="tmp", name="sf")
                        nc.scalar.activation(sf[:], pt_f[:], AF.Sigmoid, bias=bg_sb[:, 1:2])
                        nc.vector.tensor_mul(cw, cw, sf[:])
                        nc.vector.tensor_add(cw, cw, pr[:])
                    tct = ptmp.tile([128, n], F32, tag="tmp", name="tct")
                    nc.scalar.activation(tct[:], cw, AF.Tanh)
                    so = ptmp.tile([128, n], F32, tag="tmp", name="so")
                    nc.scalar.activation(so[:], pt_o[:], AF.Sigmoid, bias=bg_sb[:, 2:3])
                    nc.vector.tensor_mul(hw, so[:], tct[:])

                # zero h rows outside the global image (SAME-padding at slab edges)
                nc.vector.tensor_mul(
                    h_cur[:, 1 : 1 + t_steps, :],
                    h_cur[:, 1 : 1 + t_steps, :],
                    mtop_sb[:],
                )
                nc.vector.tensor_mul(
                    h_cur[:, base + slab : base + slab + t_steps, :],
                    h_cur[:, base + slab : base + slab + t_steps, :],
                    mbot_sb[:],
                )

                # fused output conv on this step's h (out rows are the slab
                # only). M=64 wastes half the PE array, so col-tile two 4-row
                # blocks concurrently: block A in array cols 0-63 -> PSUM
                # partitions 0-63, block B in cols 64-127 -> partitions 64-127.
                for yo in range(base, base + slab, 8):
                    yb = yo + 4
                    po = pps.tile([128, 512], F32, tag="ps", name="po")
                    k = 0
                    for dy in range(3):
                        for dx in range(3):
                            nc.tensor.matmul(
                                po[0:64, :],
                                wo_sb[:, dy * 3 + dx, :],
                                h_cur[:, yo - 1 + dy : yo + 3 + dy, dx : dx + 128],
                                start=(k == 0), stop=(k == 8),
                                tile_position=(0, 0),
                            )
                            nc.tensor.matmul(
                                po[64:128, :],
                                wo_sb[:, dy * 3 + dx, :],
                                h_cur[:, yb - 1 + dy : yb + 3 + dy, dx : dx + 128],
                                start=(k == 0), stop=(k == 8),
                                tile_position=(0, 64),
                            )
                            k += 1
                    ob = pout.tile([128, 4, 128], F32, tag="ostage", name="ob")
                    nc.scalar.activation(ob[:], po[:], AF.Identity, bias=bo_sb[:, 0:1])
                    nc.sync.dma_start(
                        out[t - 1, :, yo - base : yo - base + 4, :], ob[0:64]
                    )
                    nc.sync.dma_start(
                        out[t - 1, :, yb - base : yb - base + 4, :], ob[64:128]
                    )

    nc.compile()
    return nc


def prep_weights(W_gates, b_gates, W_out, b_out):
    wg = np.ascontiguousarray(W_gates, dtype=np.float32)  # [512, 192, 3, 3]
    wh = np.ascontiguousarray(
        wg[:, CIN:, :, :].reshape(512, 128, 9).transpose(1, 2, 0)
    )  # [128, 9, 512]
    wxp = np.ascontiguousarray(
        np.concatenate(
            [wg[:, :CIN, 0, :].transpose(1, 2, 0), wg[:, :CIN, 1, :].transpose(1, 2, 0)],
            axis=0,
        )
    )  # [128, 3, 512]
    wxc = np.ascontiguousarray(
        np.concatenate(
            [wg[:, :CIN, 2, 0].transpose(1, 0), wg[:, :CIN, 2, 1].transpose(1, 0)],
            axis=0,
        )
    )  # [128, 512]
    wx2 = np.zeros((128, 512), np.float32)
    wx2[64:] = wg[:, :CIN, 2, 2].transpose(1, 0)
    wo = np.ascontiguousarray(
        np.asarray(W_out, np.float32).reshape(64, 128, 9).transpose(1, 2, 0)
    )  # [128, 9, 64]
    bg = np.ascontiguousarray(np.asarray(b_gates, np.float32).reshape(4, 128).T)
    # out-conv bias replicated across both col-tile halves (block A / block B)
    bo = np.ascontiguousarray(
        np.tile(np.asarray(b_out, np.float32).reshape(64, 1), (2, 1))
    )
    return {
        "wh": _mm_np(wh), "wxp": _mm_np(wxp), "wxc": _mm_np(wxc),
        "wx2": _mm_np(wx2), "wo": _mm_np(wo), "bg": bg, "bo": bo,
    }


def prep_masks(r0, t_steps=T, slab=SLAB, h_img=H_FULL):
    """Row masks (1=keep, 0=zero) for the top/bottom halo bands of a slab
    starting at global row r0."""
    base = t_steps + 1
    mtop = np.zeros((128, t_steps, WP), np.float32)
    mbot = np.zeros((128, t_steps, WP), np.float32)
    for j in range(t_steps):
        r = 1 + j
        if 0 <= r0 + r - base < h_img:
            mtop[:, j, :] = 1.0
        r = base + slab + j
        if 0 <= r0 + r - base < h_img:
            mbot[:, j, :] = 1.0
    return {"mtop": _mm_np(mtop), "mbot": _mm_np(mbot)}


def prep_x(x, t_steps=T, slab=SLAB, h_img=H_FULL):
    """x: [T, B, C, H, W] -> list of per-core packed [T, 128, hbuf, WP] arrays.

    Core c = b * NSLAB + s covers global rows [slab*s, slab*s + slab).
    Partitions 0:64 hold x rows as-is, 64:128 the same rows shifted +1, so
    conv taps dy=0/1 share one matmul and dy=2 reads the shifted half.
    """
    base = t_steps + 1
    hbuf = slab + 2 * base
    nslab = h_img // slab
    x = np.asarray(x, np.float32)
    tt, nb = x.shape[0], x.shape[1]
    cores = []
    for b in range(nb):
        xpad = np.zeros((tt, CIN, h_img + 2 * base + 1, WP), np.float32)
        xpad[:, :, base : base + h_img, 1 : 1 + W] = x[:, b]
        for s in range(nslab):
            r0 = slab * s
            lower = xpad[:, :, r0 : r0 + hbuf, :]
            upper = xpad[:, :, r0 + 1 : r0 + 1 + hbuf, :]
            cores.append(_mm_np(np.concatenate([lower, upper], axis=1)))
    return cores


_NC_CACHE = {}


def _get_nc():
    key = (T, SLAB, FAST_DTYPE)
    if key not in _NC_CACHE:
        _NC_CACHE[key] = build_nc(T, SLAB)
    return _NC_CACHE[key]


def kernel(x, W_gates, b_gates, W_out, b_out):
    _import_concourse()
    from concourse.bass_utils import run_bass_kernel_spmd

    nc = _get_nc()
    wmap = prep_weights(W_gates, b_gates, W_out, b_out)
    xcores = prep_x(x)
    in_maps = []
    for c, xc in enumerate(xcores):
        s = c % NSLAB
        in_maps.append(dict(wmap, xp=xc, **prep_masks(SLAB * s)))

    trace = bool(os.environ.get("KERNEL_TRACE"))
    kwargs = {}
    if trace:
        kwargs = {"trace": True, "tmpdir": os.environ.get("KERNEL_TRACE_DIR") or None}
    res = run_bass_kernel_spmd(nc, in_maps, core_ids=list(range(N_CORES)), **kwargs)
    if trace:
        global LAST_EXEC_NS
        LAST_EXEC_NS = res.exec_time_ns
        print(f"HW exec time: {res.exec_time_ns} ns")

    out = np.empty((T, NB, CIN, H_FULL, W), np.float32)
    for c in range(N_CORES):
        b, s = divmod(c, NSLAB)
        out[:, b, :, SLAB * s : SLAB * (s + 1), :] = res.results[c]["out"]
    return out



# revision 20
# speedup vs baseline: 1.4240x; 1.1170x over previous
"""ConvLSTM (nn_BottomConvLSTM) Trainium2 Bass kernel.

Problem (hardcoded):
  x:       [T=12, B=2, C=64, H=128, W=128] f32
  W_gates: [512, 192, 3, 3] f32,  b_gates: [512] f32
  W_out:   [64, 128, 3, 3] f32,   b_out:   [64] f32
  out:     [T, B, 64, H, W] f32

Sharding: 8 cores = B(2) x H-slabs(4 x 32 rows). The T recurrence stays
on-chip per core. Each step computes exactly the core's own 32 h rows; the
3 boundary rows every step's successors need (row 31 down, rows 0-1 up)
are exchanged with slab neighbors via a per-step DRAM AllGather over the
4-core B-group. The rank-dependent gather blocks are reduced into the halo
rows with per-core one-hot masks (keeps one uniform SPMD program); the
masks are zero at the global image boundary, reproducing SAME zero padding.

Convs run as shifted matmuls accumulating in PSUM, bf16 inputs (1 col/cycle
+ FWL weight loads), fp32 accumulation. The x channels (64) are packed
twice along partitions with a +1 row shift so taps (0,dx)+(1,dx) fuse into
one K=128 matmul; (2,0)+(2,1) fuse via a +1 column-shifted packed copy;
(2,2) runs as a zero-padded K=128 matmul (a K=64 MM would break LDWEIGHTS
background-buffer pipelining). The output conv is fused per timestep and
col-tiled: two 4-row blocks run concurrently in the two PE column halves.
"""

import os
import sys

import numpy as np

T = 12
CIN = 64
HID = 128
H_FULL = 128
W = 128
NB = 2
NSLAB = 4
SLAB = H_FULL // NSLAB  # 32
WP = W + 2  # zero-padded width

HROWS = SLAB + 3  # h buffer rows: halo -1 | own 0..31 | halo 32, 33
XROWS = SLAB + 4  # x buffer rows: -1 .. 34 (lower half; upper half +1)

N_CORES = 8
LAST_EXEC_NS = None

FAST_DTYPE = os.environ.get("KERNEL_MM_DTYPE", "bfloat16")


def _mm_np(a):
    """Cast a host array to the matmul input dtype."""
    if FAST_DTYPE == "bfloat16":
        import ml_dtypes

        return np.ascontiguousarray(a.astype(ml_dtypes.bfloat16))
    return np.ascontiguousarray(a, dtype=np.float32)


def _import_concourse():
    try:
        import concourse.bass  # noqa: F401
        return
    except ImportError:
        pass
    for p in ("/opt/trn_rl_repo", "/root/.axon_site/_ro/trn_rl_repo"):
        if os.path.isdir(p) and p not in sys.path:
            sys.path.insert(0, p)
    import concourse.bass  # noqa: F401


def build_nc(t_steps=T, slab=SLAB):
    _import_concourse()
    import concourse.tile as tile
    from concourse import bacc, mybir

    F32 = mybir.dt.float32
    FMM = getattr(mybir.dt, FAST_DTYPE)
    AF = mybir.ActivationFunctionType

    nc = bacc.Bacc("TRN2", target_bir_lowering=False, debug=False)
    xp = nc.dram_tensor("xp", [t_steps, 128, XROWS, WP], FMM, kind="ExternalInput").ap()
    whd = nc.dram_tensor("wh", [128, 9, 512], FMM, kind="ExternalInput").ap()
    wxpd = nc.dram_tensor("wxp", [128, 3, 512], FMM, kind="ExternalInput").ap()
    wxcd = nc.dram_tensor("wxc", [128, 512], FMM, kind="ExternalInput").ap()
    wx2d = nc.dram_tensor("wx2", [128, 512], FMM, kind="ExternalInput").ap()
    wod = nc.dram_tensor("wo", [128, 9, 64], FMM, kind="ExternalInput").ap()
    bgd = nc.dram_tensor("bg", [128, 4], F32, kind="ExternalInput").ap()
    bod = nc.dram_tensor("bo", [128, 1], F32, kind="ExternalInput").ap()
    # one-hot gather-block masks: hm[:, j, 0, :] selects block j for halo
    # row -1; hm[:, j, 1:3, :] selects block j for halo rows 32-33. All-zero
    # at the global image boundary (SAME zero padding).
    hmd = nc.dram_tensor("hm", [128, 4, 3, WP], FMM, kind="ExternalInput").ap()
    out = nc.dram_tensor("out", [t_steps, 64, slab, W], F32, kind="ExternalOutput").ap()
    # per-step exchange bounces (collectives need Internal DRAM)
    bin_ = nc.dram_tensor("hbin", [t_steps, 128, 3, WP], FMM).ap()
    bout = nc.dram_tensor("hbout", [t_steps, 4, 128, 3, WP], FMM).ap()
    # tiny warmup-collective buffers: establishes the CC channels while the
    # weight DMAs and step-1 matmuls run, so exchange #1 isn't serialized
    # behind channel setup (trace showed a ~20us first-exchange stall)
    dwin = nc.dram_tensor("dwin", [128, 16], FMM).ap()
    dwout = nc.dram_tensor("dwout", [4, 128, 16], FMM).ap()

    groups = [[0, 1, 2, 3], [4, 5, 6, 7]]

    with tile.TileContext(nc) as tc:
        with (
            tc.tile_pool(name="pw", bufs=1) as pw,
            tc.tile_pool(name="pstate", bufs=1) as pstate,
            tc.tile_pool(name="px", bufs=4) as px,
            tc.tile_pool(name="ptmp", bufs=18) as ptmp,
            tc.tile_pool(name="prx", bufs=2) as prx,
            tc.tile_pool(name="pout", bufs=3) as pout,
            tc.tile_pool(name="pps", bufs=8, space="PSUM") as pps,
        ):
            # Warm the PE clock (HAM un-throttles after ~3.4us of activity)
            # with dummy matmuls on a zeroed tile while the weight DMAs are
            # still in flight.
            warm = pw.tile([128, 640], FMM, tag="warm", name="warm")
            nc.vector.memset(warm[:], 0)
            wps = pps.tile([128, 512], F32, tag="ps", name="warm_ps")
            for k in range(16):
                nc.tensor.matmul(
                    wps[:], warm[:, 0:128], warm[:, 128:640],
                    start=(k == 0), stop=(k == 15),
                )
            nc.gpsimd.collective_compute(
                "AllGather",
                mybir.AluOpType.bypass,
                replica_groups=groups,
                ins=[dwin],
                outs=[dwout],
            )

            wh_sb = pw.tile([128, 9, 512], FMM, tag="wh", name="wh_sb")
            wxp_sb = pw.tile([128, 3, 512], FMM, tag="wxp", name="wxp_sb")
            wxc_sb = pw.tile([128, 512], FMM, tag="wxc", name="wxc_sb")
            wx2_sb = pw.tile([128, 512], FMM, tag="wx2", name="wx2_sb")
            wo_sb = pw.tile([128, 9, 64], FMM, tag="wo", name="wo_sb")
            bg_sb = pw.tile([128, 4], F32, tag="bg", name="bg_sb")
            bo_sb = pw.tile([128, 1], F32, tag="bo", name="bo_sb")
            hm_sb = pw.tile([128, 4, 3, WP], FMM, tag="hm", name="hm_sb")

            # x-weights first: step 1 needs no h-weights, so its matmuls can
            # start as soon as the small x-weight tiles land
            for dx in range(3):
                nc.sync.dma_start(wxp_sb[:, dx, :], wxpd[:, dx, :])
            nc.sync.dma_start(wxc_sb[:], wxcd[:])
            nc.sync.dma_start(wx2_sb[:], wx2d[:])
            nc.sync.dma_start(bg_sb[:], bgd[:])
            nc.sync.dma_start(bo_sb[:], bod[:])
            nc.sync.dma_start(wo_sb[:], wod[:])
            nc.sync.dma_start(hm_sb[:], hmd[:])
            nc.sync.dma_start(wh_sb[:], whd[:])

            h_a = pstate.tile([128, HROWS, WP], FMM, tag="ha", name="h_a")
            h_b = pstate.tile([128, HROWS, WP], FMM, tag="hb", name="h_b")
            c_sb = pstate.tile([128, slab, W], F32, tag="c", name="c_sb")
            nc.vector.memset(h_a[:], 0)
            nc.vector.memset(h_b[:], 0)
            h_tiles = [h_a, h_b]

            # gate tiles: edge tiles (0, 28) early so their h rows feed the
            # exchange; one interior tile first so step t's edge matmuls
            # aren't immediately blocked on step t-1's exchange.
            tile_order = [4, 0, 28, 8, 12, 16, 20, 24]

            def emit_out_pair(tt, h_t, yo):
                """Col-tiled out-conv pair: rows yo..yo+4 (A) and +4 (B)."""
                ya, yb = yo + 1, yo + 5
                po = pps.tile([128, 512], F32, tag="ps", name="po")
                k = 0
                for dy in range(3):
                    for dx in range(3):
                        nc.tensor.matmul(
                            po[0:64, :],
                            wo_sb[:, dy * 3 + dx, :],
                            h_t[:, ya - 1 + dy : ya + 3 + dy, dx : dx + 128],
                            start=(k == 0), stop=(k == 8),
                            tile_position=(0, 0),
                        )
                        nc.tensor.matmul(
                            po[64:128, :],
                            wo_sb[:, dy * 3 + dx, :],
                            h_t[:, yb - 1 + dy : yb + 3 + dy, dx : dx + 128],
                            start=(k == 0), stop=(k == 8),
                            tile_position=(0, 64),
                        )
                        k += 1
                ob = pout.tile([128, 4, 128], F32, tag="ostage", name="ob")
                nc.scalar.activation(ob[:], po[:], AF.Identity, bias=bo_sb[:, 0:1])
                nc.sync.dma_start(out[tt - 1, :, yo : yo + 4, :], ob[0:64])
                nc.sync.dma_start(out[tt - 1, :, yo + 4 : yo + 8, :], ob[64:128])

            for t in range(1, t_steps + 1):
                h_cur = h_tiles[(t - 1) % 2]
                h_prev = h_tiles[t % 2]

                for ti, y in enumerate(tile_order):
                    r = 4
                    n = r * 128
                    xs = px.tile([128, r + 2, WP], FMM, tag="xs", name="xs")
                    nc.sync.dma_start(xs[:, :, :], xp[t - 1, :, y : y + r + 2, :])
                    # col-pair tile: lower = x rows y+1.. (dy=2), upper +1 col
                    xc = px.tile([128, r, WP], FMM, tag="xc", name="xc")
                    nc.sync.dma_start(
                        xc[0:64, :, :], xp[t - 1, 0:64, y + 2 : y + r + 2, :]
                    )
                    nc.sync.dma_start(
                        xc[64:128, :, 0 : WP - 1],
                        xp[t - 1, 0:64, y + 2 : y + r + 2, 1:WP],
                    )

                    # coc order g,i,f,o: the g-gate PSUM (longest pointwise
                    # chain) lands first; pointwise below is emitted in
                    # matching order so the strict-FIFO ACT queue never
                    # stalls on a late PSUM.
                    psums = {}
                    for coc in (3, 0, 1, 2):
                        pt = pps.tile([128, n], F32, tag="ps", name="ps")
                        psums[coc] = pt
                        mms = []
                        for dx in range(3):
                            mms.append((
                                wxp_sb[:, dx, coc * 128 : (coc + 1) * 128],
                                xs[:, 0:r, dx : dx + 128],
                            ))
                        mms.append((
                            wxc_sb[:, coc * 128 : (coc + 1) * 128],
                            xc[:, 0:r, 0:128],
                        ))
                        mms.append((
                            wx2_sb[:, coc * 128 : (coc + 1) * 128],
                            xs[:, 1 : 1 + r, 2:130],
                        ))
                        if t > 1:
                            for dy in range(3):
                                for dx in range(3):
                                    mms.append((
                                        wh_sb[:, dy * 3 + dx, coc * 128 : (coc + 1) * 128],
                                        h_prev[:, y + dy : y + dy + r, dx : dx + 128],
                                    ))
                        for k, (lhsT, rhs) in enumerate(mms):
                            nc.tensor.matmul(
                                pt[:], lhsT, rhs,
                                start=(k == 0), stop=(k == len(mms) - 1),
                            )

                    pt_i, pt_f, pt_o, pt_g = (psums[c] for c in range(4))
                    cw = c_sb[:, y : y + r, :]
                    hw = h_cur[:, y + 1 : y + 1 + r, 1:129]

                    tg = ptmp.tile([128, n], F32, tag="tmp", name="tg")
                    nc.scalar.activation(tg[:], pt_g[:], AF.Tanh, bias=bg_sb[:, 3:4])
                    si = ptmp.tile([128, n], F32, tag="tmp", name="si")
                    nc.scalar.activation(si[:], pt_i[:], AF.Sigmoid, bias=bg_sb[:, 0:1])
                    if t == 1:
                        nc.vector.tensor_mul(cw, si[:], tg[:])
                    else:
                        pr = ptmp.tile([128, n], F32, tag="tmp", name="pr")
                        nc.vector.tensor_mul(pr[:], si[:], tg[:])
                        sf = ptmp.tile([128, n], F32, tag="tmp", name="sf")
                        nc.scalar.activation(sf[:], pt_f[:], AF.Sigmoid, bias=bg_sb[:, 1:2])
                        nc.vector.tensor_mul(cw, cw, sf[:])
                        nc.vector.tensor_add(cw, cw, pr[:])
                    tct = ptmp.tile([128, n], F32, tag="tmp", name="tct")
                    nc.scalar.activation(tct[:], cw, AF.Tanh)
                    so = ptmp.tile([128, n], F32, tag="tmp", name="so")
                    nc.scalar.activation(so[:], pt_o[:], AF.Sigmoid, bias=bg_sb[:, 2:3])
                    nc.vector.tensor_mul(hw, so[:], tct[:])

                    if y == 0:
                        # rows 0,1 -> neighbor above (its halo rows 32,33);
                        # slots 1:3 match the rx->halo reduce layout below
                        nc.gpsimd.dma_start(
                            bin_[t - 1, :, 1:3, :], h_cur[:, 1:3, :]
                        )
                    elif y == slab - 4:
                        # row 31 -> neighbor below (its halo row -1); slot 0
                        nc.gpsimd.dma_start(
                            bin_[t - 1, :, 0:1, :], h_cur[:, slab : slab + 1, :]
                        )
                    if ti == 0 and t > 1:
                        # deferred edge out-conv pairs of step t-1: they read
                        # h_prev's exchange-fed halo rows, so running them
                        # here (instead of at the end of step t-1) keeps the
                        # PE queue from stalling on the exchange
                        emit_out_pair(t - 1, h_prev, 0)
                        emit_out_pair(t - 1, h_prev, slab - 8)

                # exchange the 3 edge rows within the B-group
                nc.gpsimd.collective_compute(
                    "AllGather",
                    mybir.AluOpType.bypass,
                    replica_groups=groups,
                    ins=[bin_[t - 1]],
                    outs=[bout[t - 1]],
                )
                rx = prx.tile([128, 4, 3, WP], FMM, tag="rx", name="rx")
                for j in range(4):
                    nc.sync.dma_start(rx[:, j, :, :], bout[t - 1, j, :, :, :])
                mrx = prx.tile([128, 4, 3, WP], FMM, tag="mrx", name="mrx")
                nc.vector.tensor_mul(mrx[:], rx[:], hm_sb[:])
                t01 = prx.tile([128, 3, WP], FMM, tag="t01", name="t01")
                nc.vector.tensor_add(t01[:], mrx[:, 0, :, :], mrx[:, 1, :, :])
                t23 = prx.tile([128, 3, WP], FMM, tag="t23", name="t23")
                nc.vector.tensor_add(t23[:], mrx[:, 2, :, :], mrx[:, 3, :, :])
                # halo row -1 (buf 0) from each block's slot 0; rows 32,33
                # (buf 33,34) from slots 1,2
                nc.vector.tensor_add(
                    h_cur[:, 0:1, :], t01[:, 0:1, :], t23[:, 0:1, :]
                )
                nc.vector.tensor_add(
                    h_cur[:, slab + 1 : slab + 3, :], t01[:, 1:3, :], t23[:, 1:3, :]
                )

                # interior out-conv pairs of step t; the two edge pairs
                # (which read this step's exchange-fed halo rows) are
                # deferred into step t+1's emission stream
                emit_out_pair(t, h_cur, 8)
                emit_out_pair(t, h_cur, 16)

            emit_out_pair(t_steps, h_tiles[(t_steps - 1) % 2], 0)
            emit_out_pair(t_steps, h_tiles[(t_steps - 1) % 2], slab - 8)

    nc.compile()
    return nc


def prep_weights(W_gates, b_gates, W_out, b_out):
    wg = np.ascontiguousarray(W_gates, dtype=np.float32)  # [512, 192, 3, 3]
    wh = np.ascontiguousarray(
        wg[:, CIN:, :, :].reshape(512, 128, 9).transpose(1, 2, 0)
    )  # [128, 9, 512]
    wxp = np.ascontiguousarray(
        np.concatenate(
            [wg[:, :CIN, 0, :].transpose(1, 2, 0), wg[:, :CIN, 1, :].transpose(1, 2, 0)],
            axis=0,
        )
    )  # [128, 3, 512]
    wxc = np.ascontiguousarray(
        np.concatenate(
            [wg[:, :CIN, 2, 0].transpose(1, 0), wg[:, :CIN, 2, 1].transpose(1, 0)],
            axis=0,
        )
    )  # [128, 512]
    wx2 = np.zeros((128, 512), np.float32)
    wx2[64:] = wg[:, :CIN, 2, 2].transpose(1, 0)
    wo = np.ascontiguousarray(
        np.asarray(W_out, np.float32).reshape(64, 128, 9).transpose(1, 2, 0)
    )  # [128, 9, 64]
    bg = np.ascontiguousarray(np.asarray(b_gates, np.float32).reshape(4, 128).T)
    # out-conv bias replicated across both col-tile halves (block A / block B)
    bo = np.ascontiguousarray(
        np.tile(np.asarray(b_out, np.float32).reshape(64, 1), (2, 1))
    )
    return {
        "wh": _mm_np(wh), "wxp": _mm_np(wxp), "wxc": _mm_np(wxc),
        "wx2": _mm_np(wx2), "wo": _mm_np(wo), "bg": bg, "bo": bo,
    }


def prep_hm(s):
    """One-hot gather-block masks for slab index s (group-local rank).

    hm[:, j, 0, :]   = 1 iff block j is my upper neighbor (s-1): halo row -1.
    hm[:, j, 1:3, :] = 1 iff block j is my lower neighbor (s+1): rows 32,33.
    Zero rows at the global image boundary (SAME zero padding).
    """
    hm = np.zeros((128, 4, 3, WP), np.float32)
    if s > 0:
        hm[:, s - 1, 0, :] = 1.0
    if s < NSLAB - 1:
        hm[:, s + 1, 1:3, :] = 1.0
    return {"hm": _mm_np(hm)}


def prep_x(x, t_steps=T, slab=SLAB, h_img=H_FULL):
    """x: [T, B, C, H, W] -> list of per-core packed [T, 128, XROWS, WP].

    Core c = b * NSLAB + s covers global rows [slab*s, slab*s + slab).
    Buffer row b holds x row slab*s + b - 1 (rows -1..34) in partitions
    0:64, and the same +1 row shift in partitions 64:128.
    """
    x = np.asarray(x, np.float32)
    tt, nb = x.shape[0], x.shape[1]
    cores = []
    for b in range(nb):
        xpad = np.zeros((tt, CIN, h_img + XROWS + 2, WP), np.float32)
        xpad[:, :, 2 : 2 + h_img, 1 : 1 + W] = x[:, b]
        for s in range(NSLAB):
            r0 = slab * s  # buffer row 0 = global row r0 - 1 = xpad row r0+1
            lower = xpad[:, :, r0 + 1 : r0 + 1 + XROWS, :]
            upper = xpad[:, :, r0 + 2 : r0 + 2 + XROWS, :]
            cores.append(_mm_np(np.concatenate([lower, upper], axis=1)))
    return cores


_NC_CACHE = {}


def _get_nc():
    key = (T, SLAB, FAST_DTYPE)
    if key not in _NC_CACHE:
        _NC_CACHE[key] = build_nc(T, SLAB)
    return _NC_CACHE[key]


def kernel(x, W_gates, b_gates, W_out, b_out):
    _import_concourse()
    from concourse.bass_utils import run_bass_kernel_spmd

    nc = _get_nc()
    wmap = prep_weights(W_gates, b_gates, W_out, b_out)
    xcores = prep_x(x)
    in_maps = []
    for c, xc in enumerate(xcores):
        s = c % NSLAB
        in_maps.append(dict(wmap, xp=xc, **prep_hm(s)))

    trace = bool(os.environ.get("KERNEL_TRACE"))
    kwargs = {}
    if trace:
        kwargs = {"trace": True, "tmpdir": os.environ.get("KERNEL_TRACE_DIR") or None}
    res = run_bass_kernel_spmd(nc, in_maps, core_ids=list(range(N_CORES)), **kwargs)
    if trace:
        global LAST_EXEC_NS
        LAST_EXEC_NS = res.exec_time_ns
        print(f"HW exec time: {res.exec_time_ns} ns")

    out = np.empty((T, NB, CIN, H_FULL, W), np.float32)
    for c in range(N_CORES):
        b, s = divmod(c, NSLAB)
        out[:, b, :, SLAB * s : SLAB * (s + 1), :] = res.results[c]["out"]
    return out
